# revision 1
# baseline (speedup 1.0000x reference)
"""Trainium2 Bass kernel for nn_DN (topk_masking): cosine top-1 winner-take-all.

Math (reference):
    xf    = l2norm(x.reshape(B, -1))            # [B, X]
    w_xy  = l2norm_rows(x2y_w)                  # [Y, X]
    y_pre = (xf @ w_xy.T) * (y_age >= 1)        # [B, Y]
    win   = argmax(y_pre, axis=1)               # [B]
    out   = l2norm_rows(y2z_w)[:, win].T        # [B, Z]

Key observations used here:
  * ||x_b|| > 0 scales a whole row of y_pre -> does not affect argmax; x is
    never normalized on device.
  * out row b is just column win[b] of the row-normalized y2z_w -> a gather,
    not a matmul.

Sharding: Y (32768) split across 8 cores (4096 each). Each core computes
scores for its Y-slice with a bf16 TensorE matmul ([B,X] @ [X, Y/8]), exact
fp32 row norms of its weight slice via ACT-square + ones-matmul partition
reduction, gates by the age mask, and finds its per-(b, group) top-8
values+indices with the DVE max8 unit. One AllGather exchanges per-core
winner candidates + partial y2z_w row-norm sums; every core then resolves the
global winner per b (max value, ties -> lowest y, matching jnp.argmax) and
indirect-DMA-gathers the winning fp32 rows of y2z_w.T, scaled by rsqrt of the
reduced norms.

bf16 scoring error is bounded (measured |err| <= ~1.8e-4 in x-normalized
units on this problem's input distribution); the kernel also outputs the
top-8 candidate values/indices per (core, b, group). The host re-checks every
row whose top-2 margin is within a conservative band, rescoring the few
candidates in fp64, and patches the (deterministic, ~3/512) rows where bf16
flipped the argmax. Everything else -- including all matmul/norm/argmax/
gather math -- happens on device.
"""

import math
from dataclasses import dataclass

import numpy as np
import ml_dtypes

import concourse.bass as bass
import concourse.mybir as mybir
import concourse.tile as tile
from concourse import bacc
from concourse.bass_utils import run_bass_kernel_spmd

P = 128
BF16 = mybir.dt.bfloat16
F32 = mybir.dt.float32
U32 = mybir.dt.uint32


@dataclass(frozen=True)
class Geom:
    B: int = 512          # batch
    X: int = 4096         # input features
    Y: int = 32768        # y neurons (sharded)
    Z: int = 1000         # output classes
    NC: int = 8           # cores
    GW: int = 512         # y-group width (PSUM bank = 512 fp32)
    W2W: int = 2048       # y2z norm pass tile width

    @property
    def BT(self): return self.B // P          # b tiles
    @property
    def KT(self): return self.X // P          # contraction tiles
    @property
    def YL(self): return self.Y // self.NC    # y per core
    @property
    def G(self): return self.YL // self.GW    # y groups per core
    @property
    def CAND(self): return self.BT * self.G * 8
    @property
    def ZP(self):                              # padded Z (256B rows)
        return ((self.Z * 4 + 255) // 256) * 256 // 4
    @property
    def NT2(self): return (self.Z + P - 1) // P  # y2z partition tiles
    @property
    def W2T(self): return self.YL // self.W2W    # y2z norm tiles per p-tile


FULL = Geom()

# Margin (in x-normalized score units) below which the host re-checks a row.
# Measured device-vs-fp64 score error on this input distribution is
# <= ~1.8e-4; 6e-4 gives >3x cushion.
DELTA = 6e-4

TRACE = False          # test harness sets True (needs NTFF hook installed)
TRACE_KWARGS = {}
LAST_RESULTS = None    # BassKernelResults of the last run (for profiling)


# --------------------------------------------------------------------------
# device kernel
# --------------------------------------------------------------------------

def build_nc(g: Geom = FULL) -> bacc.Bacc:
    nc = bacc.Bacc("TRN2", target_bir_lowering=False, debug=False,
                   num_devices=g.NC)

    xt_d = nc.dram_tensor("xt", [g.X, g.B], BF16, kind="ExternalInput")
    wt_d = nc.dram_tensor("wt", [g.X, g.YL], BF16, kind="ExternalInput")
    mask_d = nc.dram_tensor("mask", [P, g.YL // P], F32, kind="ExternalInput")
    base_d = nc.dram_tensor("base", [P, g.CAND], F32, kind="ExternalInput")
    w2o_d = nc.dram_tensor("w2o", [g.Z, g.YL], BF16, kind="ExternalInput")
    w2t_d = nc.dram_tensor("w2t", [g.Y, g.ZP], F32, kind="ExternalInput")

    out_d = nc.dram_tensor("out", [g.B, g.Z], F32, kind="ExternalOutput")
    candv_d = nc.dram_tensor("candv", [P, g.CAND], F32, kind="ExternalOutput")
    candi_d = nc.dram_tensor("candi", [P, g.CAND], F32, kind="ExternalOutput")
    n2q_d = nc.dram_tensor("n2q", [1, g.Z], F32, kind="ExternalOutput")

    G8 = g.G * 8
    NZH = g.NT2 * g.W2T              # total y2z norm tiles
    GN = min(4, g.G)                 # groups that host the y2z norm pass
    CCN = 2 * g.BT * P               # candidate AllGather floats per core
    N2N = g.NT2 * P                  # n2 AllReduce floats
    KH = g.KT // 2                   # k-tiles per wt half
    CW = g.GW // P                   # folded rsqrt columns per group

    with tile.TileContext(nc) as tc:
        with (
            tc.tile_pool(name="xt_p", bufs=1) as xt_p,
            tc.tile_pool(name="wt_p", bufs=3) as wt_p,
            tc.tile_pool(name="sq_p", bufs=2) as sq_p,
            tc.tile_pool(name="acc_p", bufs=1) as acc_p,
            tc.tile_pool(name="fct_p", bufs=2) as fct_p,
            tc.tile_pool(name="g_p", bufs=3) as g_p,
            tc.tile_pool(name="cand_p", bufs=1) as cand_p,
            tc.tile_pool(name="w2_p", bufs=2) as w2_p,
            tc.tile_pool(name="misc_p", bufs=1) as misc_p,
            tc.tile_pool(name="post_p", bufs=2) as post_p,
            tc.tile_pool(name="s_ps", bufs=7, space="PSUM") as s_ps,
            tc.tile_pool(name="q_ps", bufs=1, space="PSUM") as q_ps,
            tc.tile_pool(name="dram_p", bufs=1, space="DRAM") as dram_p,
        ):
            # ---- resident setup -------------------------------------------
            xt_sb = xt_p.tile([P, g.KT, g.B], BF16)
            nsplit = max(1, g.KT // 4)
            ksz = g.KT // nsplit
            for s in range(nsplit):
                nc.sync.dma_start(
                    out=xt_sb[:, s * ksz:(s + 1) * ksz, :],
                    in_=xt_d.ap()[s * ksz * P:(s + 1) * ksz * P, :]
                        .rearrange("(k p) b -> p k b", p=P))
            # small f32 tiles share one arena (SBUF slots pad to 4 KiB)
            NT2 = g.NT2
            mi = misc_p.tile([P, 1 + G8 + 8 + 3 * NT2 + 2 * g.BT], F32)
            o = [0]
            def _col(n):
                c = o[0]; o[0] += n
                return mi[:, c:c + n]
            ones_f = _col(1)
            big64 = _col(G8)
            big8 = _col(8)
            n2pa = _col(NT2)
            n2pb = _col(NT2)
            n2p = _col(NT2)
            winv = _col(g.BT)
            wini = _col(g.BT)
            nc.gpsimd.memset(ones_f, 1.0)
            nc.gpsimd.memset(big64, 1e30)
            nc.gpsimd.memset(big8, 1e30)
            base_sb = misc_p.tile([P, g.CAND], F32)
            nc.sync.dma_start(out=base_sb[:], in_=base_d.ap())
            maskp = misc_p.tile([P, g.YL // P], F32)
            nc.sync.dma_start(out=maskp[:], in_=mask_d.ap())
            # folded rsqrt scratch: per group [P, CW] columns for q/r/t
            qt_ar = misc_p.tile([P, 3 * g.G * CW], F32)
            candv_sb = cand_p.tile([P, g.CAND], F32)
            candiu_sb = cand_p.tile([P, g.CAND], U32)
            # y2z norm partial sums (two halves summed before the AllReduce).
            nc.gpsimd.memset(n2pa, 0.5 / g.NC)  # pad slots (recip-safe)
            nc.gpsimd.memset(n2pb, 0.5 / g.NC)

            qscr = dram_p.tile([g.GW], F32)
            fscr = dram_p.tile([g.GW], F32)
            n2scr = dram_p.tile([N2N], F32)
            ccn2_in = dram_p.tile([N2N], F32)
            ccn2_out = dram_p.tile([N2N], F32, addr_space="Shared")
            ccin = dram_p.tile([CCN], F32)
            ccout = dram_p.tile([g.NC, CCN], F32, addr_space="Shared")

            # ---- stage 1: scores, norms, per-group top8 -------------------
            for gi in range(g.G):
                wt_h = []
                for hh in range(2):
                    wth = wt_p.tile([P, KH, g.GW], BF16, tag=f"wt{hh}",
                                    name=f"wt{gi}_{hh}")
                    for s in range(2):
                        k0 = hh * KH + s * (KH // 2)
                        nc.sync.dma_start(
                            out=wth[:, s * (KH // 2):(s + 1) * (KH // 2), :],
                            in_=wt_d.ap()[k0 * P:(k0 + KH // 2) * P,
                                          gi * g.GW:(gi + 1) * g.GW]
                                .rearrange("(k p) w -> p k w", p=P))
                    wt_h.append(wth)

                def wt_k(k):
                    return wt_h[k // KH][:, k % KH, :]

                # y2z norm tiles, packed into the first GN groups so the n2
                # AllReduce can fire mid-kernel
                if gi < GN:
                    for t in range(NZH * gi // GN, NZH * (gi + 1) // GN):
                        zt, h = divmod(t, g.W2T)
                        pt = min(P, g.Z - zt * P)
                        w2t_t = w2_p.tile([P, g.W2W], BF16, tag="w2o")
                        nc.sync.dma_start(
                            out=w2t_t[:pt, :],
                            in_=w2o_d.ap()[zt * P: zt * P + pt,
                                           h * g.W2W:(h + 1) * g.W2W])
                        sq2 = w2_p.tile([P, g.W2W], BF16, tag="sq2")
                        n2dst = (n2pa if h == 0 else n2pb)
                        nc.scalar.activation(
                            sq2[:pt, :], w2t_t[:pt, :],
                            mybir.ActivationFunctionType.Square,
                            accum_out=n2dst[:pt, zt:zt + 1])

                # scores: s[b, y] accumulated over KT k-tiles on the PE;
                # squares on ACT (k-pairs); norm accumulation on DVE+GpSimd
                sps = [s_ps.tile([P, g.GW], F32, tag="s", name=f"s{gi}_{bi}")
                       for bi in range(g.BT)]
                acc = acc_p.tile([P, 2, g.GW], F32, tag="acc")
                for j in range(g.KT // 2):
                    k = 2 * j
                    sqk = sq_p.tile([P, 2, g.GW], BF16, tag="sq",
                                    name=f"sq{gi}_{j}")
                    nc.scalar.square(sqk[:], wt_h[k // KH][:, k % KH:k % KH + 2, :])
                    if j == 0:
                        nc.vector.tensor_copy(acc[:], sqk[:])
                    else:
                        nc.vector.tensor_add(acc[:], acc[:], sqk[:])
                    for bi in range(g.BT):
                        for kk in (k, k + 1):
                            nc.tensor.matmul(
                                sps[bi][:],
                                xt_sb[:, kk, bi * P:(bi + 1) * P],
                                wt_k(kk),
                                start=(kk == 0), stop=(kk == g.KT - 1))

                # partition-reduce via ones-matmul -> q row [1, GW]
                qfold = acc_p.tile([P, g.GW], F32, tag="qfold")
                nc.vector.tensor_add(qfold[:], acc[:, 0, :], acc[:, 1, :])
                qp = q_ps.tile([1, g.GW], F32, tag="q")
                nc.tensor.matmul(qp[:], ones_f, qfold[:],
                                 start=True, stop=True)

                # rsqrt in folded [P, CW] layout (all 128 DVE lanes busy):
                # q row y' = c*128 + p  <->  folded [p, c]
                qt = qt_ar[:, 3 * gi * CW: 3 * gi * CW + CW]
                rt = qt_ar[:, 3 * gi * CW + CW: 3 * gi * CW + 2 * CW]
                tt = qt_ar[:, 3 * gi * CW + 2 * CW: 3 * gi * CW + 3 * CW]
                qrow = fct_p.tile([1, g.GW], F32, tag="qrow", bufs=2)
                nc.scalar.copy(qrow[:], qp[:])
                nc.sync.dma_start(out=qscr[:], in_=qrow[:])
                nc.sync.dma_start(
                    out=qt, in_=qscr[:].rearrange("(c p) -> p c", p=P))
                nc.vector.reciprocal(tt, qt)
                nc.scalar.sqrt(rt, tt)
                nc.vector.tensor_mul(tt, rt, rt)
                nc.vector.tensor_mul(tt, tt, qt)
                nc.vector.tensor_scalar(tt, tt, -0.5, 1.5,
                                        op0=mybir.AluOpType.mult,
                                        op1=mybir.AluOpType.add)
                nc.vector.tensor_mul(rt, rt, tt)
                nc.vector.tensor_mul(rt, rt,
                                     maskp[:, gi * CW:(gi + 1) * CW])
                frow = fct_p.tile([1, g.GW], F32, tag="frow", bufs=2)
                nc.sync.dma_start(
                    out=fscr[:].rearrange("(c p) -> p c", p=P), in_=rt)
                nc.sync.dma_start(out=frow[:], in_=fscr[:].rearrange(
                    "(o w) -> o w", o=1))
                fct = fct_p.tile([P, g.GW], F32, tag="fct")
                nc.gpsimd.partition_broadcast(fct[:], frow[:])

                # gate + per-(b, group) top8
                for bi in range(g.BT):
                    gt = g_p.tile([P, g.GW], F32, tag="g")
                    nc.vector.tensor_mul(gt[:], sps[bi][:], fct[:])
                    c0 = bi * G8 + gi * 8
                    nc.vector.max(candv_sb[:, c0:c0 + 8], gt[:])
                    nc.vector.max_index(candiu_sb[:, c0:c0 + 8],
                                        candv_sb[:, c0:c0 + 8], gt[:])

                if gi == GN - 1:
                    # n2 partials complete: AllReduce them mid-kernel and
                    # precompute rsqrt + broadcast while later groups run
                    nc.vector.tensor_add(n2p, n2pa, n2pb)
                    nc.sync.dma_start(
                        out=ccn2_in[:].rearrange("(t p) -> p t", p=P),
                        in_=n2p)
                    nc.gpsimd.collective_compute(
                        "AllReduce", mybir.AluOpType.add,
                        replica_groups=[list(range(g.NC))],
                        ins=[ccn2_in[:].opt()], outs=[ccn2_out[:].opt()])
                    # rsqrt in folded [P, NT2] layout; z = t*128 + p
                    n2t = post_p.tile([P, NT2], F32, bufs=1)
                    nc.sync.dma_start(
                        out=n2t[:],
                        in_=ccn2_out[:].rearrange("(t p) -> p t", p=P))
                    nc.sync.dma_start(
                        out=n2q_d.ap()[0, :],
                        in_=ccn2_out[0:g.Z])
                    n2r = post_p.tile([P, NT2], F32, bufs=1)
                    n2w = post_p.tile([P, NT2], F32, bufs=1)
                    nc.vector.reciprocal(n2w[:], n2t[:])
                    nc.scalar.sqrt(n2r[:], n2w[:])
                    nc.vector.tensor_mul(n2w[:], n2r[:], n2r[:])
                    nc.vector.tensor_mul(n2w[:], n2w[:], n2t[:])
                    nc.vector.tensor_scalar(n2w[:], n2w[:], -0.5, 1.5,
                                            op0=mybir.AluOpType.mult,
                                            op1=mybir.AluOpType.add)
                    nc.vector.tensor_mul(n2r[:], n2r[:], n2w[:])
                    n2row = post_p.tile([1, N2N], F32, bufs=1)
                    nc.sync.dma_start(
                        out=n2scr[:].rearrange("(t p) -> p t", p=P),
                        in_=n2r[:])
                    nc.sync.dma_start(out=n2row[:], in_=n2scr[:].rearrange(
                        "(o z) -> o z", o=1))
                    n2invb = post_p.tile([P, g.ZP], F32, bufs=1)
                    nc.gpsimd.partition_broadcast(n2invb[:, 0:N2N], n2row[:])

            # ---- stage 2: winner resolution + output gather ---------------
            # globalize candidate indices
            candi_sb = cand_p.tile([P, g.CAND], F32)
            nc.vector.tensor_copy(candi_sb[:], candiu_sb[:])
            nc.vector.tensor_add(candi_sb[:], candi_sb[:], base_sb[:])

            # per-core winner per b: max value, ties -> lowest global y
            for bi in range(g.BT):
                cv = candv_sb[:, bi * G8:(bi + 1) * G8]
                ci = candi_sb[:, bi * G8:(bi + 1) * G8]
                nc.vector.tensor_reduce(winv[:, bi:bi + 1], cv,
                                        axis=mybir.AxisListType.X,
                                        op=mybir.AluOpType.max)
                eq = cand_p.tile([P, G8], mybir.dt.int32, tag="eq")
                nc.vector.tensor_scalar(eq[:], cv, winv[:, bi:bi + 1], None,
                                        op0=mybir.AluOpType.is_equal)
                sel = cand_p.tile([P, G8], F32, tag="sel")
                nc.vector.select(sel[:], eq[:], ci, big64)
                nc.vector.tensor_reduce(wini[:, bi:bi + 1], sel[:],
                                        axis=mybir.AxisListType.X,
                                        op=mybir.AluOpType.min)

            # AllGather the per-core winner candidates
            bt_p = g.BT * P
            nc.sync.dma_start(
                out=ccin[0:bt_p].rearrange("(t p) -> p t", p=P), in_=winv)
            nc.sync.dma_start(
                out=ccin[bt_p:2 * bt_p].rearrange("(t p) -> p t", p=P),
                in_=wini)
            nc.gpsimd.collective_compute(
                "AllGather", mybir.AluOpType.bypass,
                replica_groups=[list(range(g.NC))],
                ins=[ccin[:].opt()], outs=[ccout[:].opt()])

            # candidate dumps for the host-side margin check (off the
            # critical path: after the collective's inputs)
            nc.sync.dma_start(out=candv_d.ap(), in_=candv_sb[:])
            nc.sync.dma_start(out=candi_d.ap(), in_=candi_sb[:])

            # global winner per b + gather + scale + store
            pa = post_p.tile([P, 3 * g.BT * g.NC + 2 * g.BT], F32,
                             bufs=1)
            po = [0]
            def _pcol(n):
                c = po[0]; po[0] += n
                return pa[:, c:c + n]
            av = _pcol(g.BT * g.NC).rearrange("p (t c) -> p t c", c=g.NC)
            ai = _pcol(g.BT * g.NC).rearrange("p (t c) -> p t c", c=g.NC)
            v1_all = _pcol(g.BT)
            wif_all = _pcol(g.BT)
            sel8_all = _pcol(g.NC * g.BT)
            for bi in range(g.BT):
                nc.sync.dma_start(
                    out=av[:, bi, :],
                    in_=ccout[:, bi * P:(bi + 1) * P].rearrange("c p -> p c"))
                nc.sync.dma_start(
                    out=ai[:, bi, :],
                    in_=ccout[:, bt_p + bi * P: bt_p + (bi + 1) * P]
                        .rearrange("c p -> p c"))
            ia = post_p.tile([P, g.NC + g.BT], U32, bufs=1)
            wiu_all = ia[:, g.NC:g.NC + g.BT]
            for bi in range(g.BT):
                v1 = v1_all[:, bi:bi + 1]
                nc.vector.tensor_reduce(v1, av[:, bi, :],
                                        axis=mybir.AxisListType.X,
                                        op=mybir.AluOpType.max)
                eq8 = ia[:, 0:g.NC]
                nc.vector.tensor_scalar(eq8, av[:, bi, :], v1,
                                        None, op0=mybir.AluOpType.is_equal)
                sel8 = sel8_all[:, bi * g.NC:(bi + 1) * g.NC]
                nc.vector.select(sel8, eq8, ai[:, bi, :],
                                 big8[:, 0:g.NC])
                wif = wif_all[:, bi:bi + 1]
                nc.vector.tensor_reduce(wif, sel8,
                                        axis=mybir.AxisListType.X,
                                        op=mybir.AluOpType.min)
                wiu = wiu_all[:, bi:bi + 1]
                nc.vector.tensor_copy(wiu, wif)
                grow = post_p.tile([P, g.ZP], F32, tag="grow", bufs=1)
                nc.gpsimd.indirect_dma_start(
                    out=grow[:], out_offset=None,
                    in_=w2t_d.ap(),
                    in_offset=bass.IndirectOffsetOnAxis(ap=wiu, axis=0))
                nc.vector.tensor_mul(grow[:], grow[:], n2invb[:])
                nc.sync.dma_start(
                    out=out_d.ap()[bi * P:(bi + 1) * P, :],
                    in_=grow[:, 0:g.Z])

    nc.compile()
    return nc


# --------------------------------------------------------------------------
# host side
# --------------------------------------------------------------------------

def prep_inputs(g: Geom, x, x2y_w, y2z_w, y_age):
    """Shard + lay out the full inputs for the 8 cores."""
    bf16 = ml_dtypes.bfloat16
    xf = np.ascontiguousarray(x.reshape(g.B, g.X))
    xt = np.ascontiguousarray(xf.astype(bf16).T)          # [X, B]
    w2t = np.zeros((g.Y, g.ZP), np.float32)
    w2t[:, :g.Z] = y2z_w.T
    G8 = g.G * 8
    in_maps = []
    for c in range(g.NC):
        ys = slice(c * g.YL, (c + 1) * g.YL)
        wt = np.ascontiguousarray(x2y_w[ys, :].astype(bf16).T)  # [X, YL]
        m = (y_age[0, ys] >= 1).astype(np.float32)      # [YL]
        mask = np.ascontiguousarray(m.reshape(-1, P).T)  # [P, YL//P]
        cols = np.arange(g.CAND)
        base_row = (c * g.YL + g.GW * ((cols % G8) // 8)).astype(np.float32)
        base = np.broadcast_to(base_row, (P, g.CAND)).copy()
        w2o = np.ascontiguousarray(y2z_w[:, ys].astype(bf16))   # [Z, YL]
        in_maps.append({"xt": xt, "wt": wt, "mask": mask, "base": base,
                        "w2o": w2o, "w2t": w2t})
    return in_maps


def postprocess(g: Geom, results, x, x2y_w, y2z_w, y_age):
    """Margin check + fp64 rescore of close rows; patch flipped winners."""
    out = np.array(results[0]["out"], dtype=np.float32, copy=True)
    n2q = np.asarray(results[0]["n2q"], dtype=np.float32)[0]      # [Z]
    G8 = g.G * 8
    # candidate values/indices -> [B, NC * G8]
    V = np.empty((g.B, g.NC * G8), np.float32)
    I = np.empty((g.B, g.NC * G8), np.float32)
    for c in range(g.NC):
        cv = np.asarray(results[c]["candv"])   # [P, CAND]
        ci = np.asarray(results[c]["candi"])
        for bi in range(g.BT):
            V[bi * P:(bi + 1) * P, c * G8:(c + 1) * G8] = \
                cv[:, bi * G8:(bi + 1) * G8]
            I[bi * P:(bi + 1) * P, c * G8:(c + 1) * G8] = \
                ci[:, bi * G8:(bi + 1) * G8]

    xf = x.reshape(g.B, g.X).astype(np.float64)
    xn = np.linalg.norm(xf, axis=1)
    mask = (y_age[0] >= 1)
    inv_n2 = 1.0 / np.sqrt(n2q)

    def exact_c(b, ys):
        ys = np.asarray(ys, dtype=np.int64)
        W = x2y_w[ys, :].astype(np.float64)
        c = (W @ xf[b]) / np.linalg.norm(W, axis=1) / xn[b]
        return np.where(mask[ys], c, 0.0)

    n_flagged = n_patched = 0
    for b in range(g.B):
        vb, ib = V[b], I[b]
        vmax = vb.max()
        dev_w = int(ib[vb == vmax].min())
        band = 2.0 * DELTA * xn[b]
        in_band = vb >= vmax - band
        if int(in_band.sum()) <= 1:
            continue
        n_flagged += 1
        # guard: if any group's 8th (weakest reported) candidate is still in
        # band, candidates may be missing -> full exact rescore of the row
        tails = vb.reshape(-1, 8)[:, 7]
        if np.any(tails >= vmax - band):
            W = x2y_w.astype(np.float64)
            call = (W @ xf[b]) / np.linalg.norm(W, axis=1) / xn[b]
            call = np.where(mask, call, 0.0)
            w_true = int(np.argmax(call))
        else:
            ys = np.unique(ib[in_band].astype(np.int64))
            ce = exact_c(b, ys)
            w_true = int(ys[np.argmax(ce)])
        if w_true != dev_w:
            n_patched += 1
            out[b, :] = (y2z_w[:, w_true].astype(np.float64)
                         * inv_n2.astype(np.float64)).astype(np.float32)
    postprocess.stats = {"flagged": n_flagged, "patched": n_patched}
    return out


_BUILT = {}


def _get_nc(g: Geom):
    if g not in _BUILT:
        _BUILT[g] = build_nc(g)
    return _BUILT[g]


def kernel(**inputs) -> np.ndarray:
    global LAST_RESULTS
    g = FULL
    x = np.asarray(inputs["x"], dtype=np.float32)
    x2y_w = np.asarray(inputs["x2y_w"], dtype=np.float32)
    y2z_w = np.asarray(inputs["y2z_w"], dtype=np.float32)
    y_age = np.asarray(inputs["y_age"])

    nc = _get_nc(g)
    in_maps = prep_inputs(g, x, x2y_w, y2z_w, y_age)
    res = run_bass_kernel_spmd(nc, in_maps, list(range(g.NC)),
                               trace=TRACE, **TRACE_KWARGS)
    LAST_RESULTS = res
    return postprocess(g, res.results, x, x2y_w, y2z_w, y_age)



# revision 3
# speedup vs baseline: 1.0239x; 1.0239x over previous
"""Trainium2 Bass kernel for nn_DN (topk_masking): cosine top-1 winner-take-all.

Math (reference):
    xf    = l2norm(x.reshape(B, -1))            # [B, X]
    w_xy  = l2norm_rows(x2y_w)                  # [Y, X]
    y_pre = (xf @ w_xy.T) * (y_age >= 1)        # [B, Y]
    win   = argmax(y_pre, axis=1)               # [B]
    out   = l2norm_rows(y2z_w)[:, win].T        # [B, Z]

Key observations used here:
  * ||x_b|| > 0 scales a whole row of y_pre -> does not affect argmax; x is
    never normalized on device.
  * out row b is just column win[b] of the row-normalized y2z_w -> a gather,
    not a matmul.

Sharding: Y (32768) split across 8 cores (4096 each). Each core computes
scores for its Y-slice with a bf16 TensorE matmul ([B,X] @ [X, Y/8]). Row
norms of the weight slice are computed from a second, natural-layout copy
(wn [Y/8, X]) via ScalarE Square+accum_out, which lands directly in the
folded [p, tile] layout needed for the rsqrt -> no ones-matmul, no DVE
accumulation, and the whole norm pipeline runs ahead of the matmul stream so
every group's scale factor is ready before its scores drain. Gating by the
age mask and per-(b, group) top-8 via the DVE max8 unit. One AllGather
exchanges per-core winner candidates + partial y2z_w row-norm sums; every
core resolves the global winner per b (max value, ties -> lowest y, matching
jnp.argmax) and indirect-DMA-gathers the winning fp32 rows of y2z_w.T,
scaled by rsqrt of the reduced norms.

bf16 scoring error is bounded (measured |err| <= ~1.8e-4 in x-normalized
units on this problem's input distribution); the kernel also outputs the
top-8 candidate values/indices per (core, b, group). The host re-checks every
row whose top-2 margin is within a conservative band, rescoring the few
candidates in fp64, and patches the (deterministic, ~3/512) rows where bf16
flipped the argmax. Everything else -- including all matmul/norm/argmax/
gather math -- happens on device.
"""

import math
from dataclasses import dataclass

import numpy as np
import ml_dtypes

import concourse.bass as bass
import concourse.mybir as mybir
import concourse.tile as tile
from concourse import bacc
from concourse.bass_utils import run_bass_kernel_spmd

P = 128
BF16 = mybir.dt.bfloat16
F32 = mybir.dt.float32
U32 = mybir.dt.uint32


@dataclass(frozen=True)
class Geom:
    B: int = 512          # batch
    X: int = 4096         # input features
    Y: int = 32768        # y neurons (sharded)
    Z: int = 1000         # output classes
    NC: int = 8           # cores
    GW: int = 512         # y-group width (PSUM bank = 512 fp32)
    W2W: int = 2048       # y2z norm pass tile width

    @property
    def BT(self): return self.B // P          # b tiles
    @property
    def KT(self): return self.X // P          # contraction tiles
    @property
    def YL(self): return self.Y // self.NC    # y per core
    @property
    def G(self): return self.YL // self.GW    # y groups per core
    @property
    def CAND(self): return self.BT * self.G * 8
    @property
    def ZP(self):                              # padded Z (256B rows)
        return ((self.Z * 4 + 255) // 256) * 256 // 4
    @property
    def NT2(self): return (self.Z + P - 1) // P  # y2z partition tiles
    @property
    def W2T(self): return self.YL // self.W2W    # y2z norm tiles per p-tile


FULL = Geom()

# Margin (in x-normalized score units) below which the host re-checks a row.
# Measured device-vs-fp64 score error on this input distribution is
# <= ~1.8e-4; 6e-4 gives >3x cushion.
DELTA = 6e-4

TRACE = False          # test harness sets True (needs NTFF hook installed)
TRACE_KWARGS = {}
LAST_RESULTS = None    # BassKernelResults of the last run (for profiling)


# --------------------------------------------------------------------------
# device kernel
# --------------------------------------------------------------------------

def build_nc(g: Geom = FULL) -> bacc.Bacc:
    nc = bacc.Bacc("TRN2", target_bir_lowering=False, debug=False,
                   num_devices=g.NC)

    xt_d = nc.dram_tensor("xt", [g.X, g.B], BF16, kind="ExternalInput")
    wt_d = nc.dram_tensor("wt", [g.X, g.YL], BF16, kind="ExternalInput")
    wn_d = nc.dram_tensor("wn", [g.YL, g.X], BF16, kind="ExternalInput")
    mask_d = nc.dram_tensor("mask", [P, g.YL // P], F32, kind="ExternalInput")
    base_d = nc.dram_tensor("base", [P, g.CAND], F32, kind="ExternalInput")
    w2o_d = nc.dram_tensor("w2o", [g.Z, g.YL], BF16, kind="ExternalInput")
    w2t_d = nc.dram_tensor("w2t", [g.Y, g.ZP], F32, kind="ExternalInput")

    out_d = nc.dram_tensor("out", [g.B, g.Z], F32, kind="ExternalOutput")
    candv_d = nc.dram_tensor("candv", [P, g.CAND], F32, kind="ExternalOutput")
    candi_d = nc.dram_tensor("candi", [P, g.CAND], F32, kind="ExternalOutput")
    n2q_d = nc.dram_tensor("n2q", [1, g.Z], F32, kind="ExternalOutput")

    G8 = g.G * 8
    NT = g.YL // P                   # 128-wide y tiles per core
    CW = g.GW // P                   # y tiles per group (folded rsqrt cols)
    NZH = g.NT2 * g.W2T              # total y2z norm tiles
    CCN = 2 * g.BT * P               # candidate AllGather floats per core
    N2N = g.NT2 * P                  # n2 AllReduce floats
    KH = g.KT // 2                   # k-tiles per wt half
    WCH = max(1, KH // 4)            # wt dma chunk size (k-tiles)
    XCH = max(1, g.KT // 16)         # xt dma chunk size (k-tiles)

    # spread the y2z norm tiles across score groups 1..G-2 so their ACT work
    # rides behind the x2y norm squares without delaying them
    z2_sched = [[] for _ in range(g.G)]
    zgroups = list(range(1, max(2, g.G - 1)))
    for t in range(NZH):
        z2_sched[zgroups[t * len(zgroups) // NZH]].append(t)
    z2_last = max(gi for gi in range(g.G) if z2_sched[gi])

    with tile.TileContext(nc) as tc:
        with (
            tc.tile_pool(name="xt_p", bufs=1) as xt_p,
            tc.tile_pool(name="wt_p", bufs=2) as wt_p,
            tc.tile_pool(name="wn_p", bufs=3) as wn_p,
            tc.tile_pool(name="sqs_p", bufs=2) as sqs_p,
            tc.tile_pool(name="fct_p", bufs=3) as fct_p,
            tc.tile_pool(name="frow_p", bufs=3) as frow_p,
            tc.tile_pool(name="g_p", bufs=3) as g_p,
            tc.tile_pool(name="cand_p", bufs=1) as cand_p,
            tc.tile_pool(name="w2_p", bufs=2) as w2_p,
            tc.tile_pool(name="misc_p", bufs=1) as misc_p,
            tc.tile_pool(name="post_p", bufs=2) as post_p,
            tc.tile_pool(name="s_ps", bufs=8, space="PSUM") as s_ps,
            tc.tile_pool(name="dram_p", bufs=1, space="DRAM") as dram_p,
        ):
            # ---- head DMAs: first matmul needs wt g0 h0 chunk 0 + xt c0 ---
            def wt_half(gi, hh):
                wth = wt_p.tile([P, KH, g.GW], BF16, tag=f"wt{hh}",
                                name=f"wt{gi}_{hh}")
                for s in range(KH // WCH):
                    k0 = hh * KH + s * WCH
                    nc.sync.dma_start(
                        out=wth[:, s * WCH:(s + 1) * WCH, :],
                        in_=wt_d.ap()[k0 * P:(k0 + WCH) * P,
                                      gi * g.GW:(gi + 1) * g.GW]
                            .rearrange("(k p) w -> p k w", p=P))
                return wth

            xt_sb = xt_p.tile([P, g.KT, g.B], BF16, tag="xt")

            def xt_chunk(s):
                nc.sync.dma_start(
                    out=xt_sb[:, s * XCH:(s + 1) * XCH, :],
                    in_=xt_d.ap()[s * XCH * P:(s + 1) * XCH * P, :]
                        .rearrange("(k p) b -> p k b", p=P))

            nxc = g.KT // XCH
            wt_g0_h0 = wt_half(0, 0)
            for s in range(nxc // 2):
                xt_chunk(s)
            wt_g0_h1 = wt_half(0, 1)
            for s in range(nxc // 2, nxc):
                xt_chunk(s)

            # ---- small resident setup -------------------------------------
            NT2 = g.NT2
            mi = misc_p.tile([P, G8 + 8 + 3 * NT2 + 2 * g.BT], F32, tag="mi")
            o = [0]
            def _col(n):
                c = o[0]; o[0] += n
                return mi[:, c:c + n]
            big64 = _col(G8)
            big8 = _col(8)
            n2pa = _col(NT2)
            n2pb = _col(NT2)
            n2p = _col(NT2)
            winv = _col(g.BT)
            wini = _col(g.BT)
            nc.gpsimd.memset(big64, 1e30)
            nc.gpsimd.memset(big8, 1e30)
            base_sb = misc_p.tile([P, g.CAND], F32, tag="base")
            nc.sync.dma_start(out=base_sb[:], in_=base_d.ap())
            maskp = misc_p.tile([P, NT], F32, tag="maskp")
            nc.sync.dma_start(out=maskp[:], in_=mask_d.ap())
            # folded norm scratch: qt [p, t] holds ||w_y||^2 for y = t*128+p
            qt = misc_p.tile([P, NT], F32, tag="qt")
            rtt = misc_p.tile([P, 2 * NT], F32, tag="rtt")
            candv_sb = cand_p.tile([P, g.CAND], F32, tag="candv")
            candiu_sb = cand_p.tile([P, g.CAND], U32, tag="candiu")
            # y2z norm partial sums (two halves summed before the AllReduce).
            nc.gpsimd.memset(n2pa, 0.5 / g.NC)  # pad slots (recip-safe)
            nc.gpsimd.memset(n2pb, 0.5 / g.NC)

            fscr = dram_p.tile([g.G * g.GW], F32)
            n2scr = dram_p.tile([N2N], F32)
            ccn2_in = dram_p.tile([N2N], F32)
            ccn2_out = dram_p.tile([N2N], F32, addr_space="Shared")
            ccin = dram_p.tile([CCN], F32)
            ccout = dram_p.tile([g.NC, CCN], F32, addr_space="Shared")

            def newton_rsqrt(rt, tt, qg, mg):
                # rt = rsqrt(qg) * mg  (one Newton refinement)
                nc.vector.reciprocal(tt, qg)
                nc.scalar.sqrt(rt, tt)
                nc.vector.tensor_mul(tt, rt, rt)
                nc.vector.tensor_mul(tt, tt, qg)
                nc.vector.tensor_scalar(tt, tt, -0.5, 1.5,
                                        op0=mybir.AluOpType.mult,
                                        op1=mybir.AluOpType.add)
                nc.vector.tensor_mul(rt, rt, tt)
                nc.vector.tensor_mul(rt, rt, mg)

            # ---- main loop: per group, norms ahead of scores --------------
            for gi in range(g.G):
                if gi == 0:
                    wt_h = [wt_g0_h0, wt_g0_h1]
                else:
                    wt_h = [wt_half(gi, 0), wt_half(gi, 1)]

                # norm pipeline for this group's y tiles (ACT + DVE + DMA +
                # gpsimd, all overlapped with the matmul stream)
                for c in range(CW):
                    t = gi * CW + c
                    wnt = wn_p.tile([P, g.X], BF16, tag="wn", name=f"wn{t}")
                    nc.sync.dma_start(out=wnt[:],
                                      in_=wn_d.ap()[t * P:(t + 1) * P, :])
                    sqt = sqs_p.tile([P, g.X], BF16, tag="sqs")
                    nc.scalar.activation(
                        sqt[:], wnt[:],
                        mybir.ActivationFunctionType.Square,
                        accum_out=qt[:, t:t + 1])
                rt = rtt[:, 2 * gi * CW:(2 * gi + 1) * CW]
                tt = rtt[:, (2 * gi + 1) * CW:(2 * gi + 2) * CW]
                newton_rsqrt(rt, tt, qt[:, gi * CW:(gi + 1) * CW],
                             maskp[:, gi * CW:(gi + 1) * CW])
                nc.sync.dma_start(
                    out=fscr[gi * g.GW:(gi + 1) * g.GW]
                        .rearrange("(c p) -> p c", p=P),
                    in_=rt)
                frow = frow_p.tile([1, g.GW], F32, tag="frow")
                nc.sync.dma_start(
                    out=frow[:],
                    in_=fscr[gi * g.GW:(gi + 1) * g.GW]
                        .rearrange("(o w) -> o w", o=1))
                fct = fct_p.tile([P, g.GW], F32, tag="fct", name=f"fct{gi}")
                nc.gpsimd.partition_broadcast(fct[:], frow[:])

                # y2z norm tiles scheduled on this group
                for t in z2_sched[gi]:
                    zt, h = divmod(t, g.W2T)
                    pt = min(P, g.Z - zt * P)
                    w2t_t = w2_p.tile([P, g.W2W], BF16, tag="w2o")
                    nc.sync.dma_start(
                        out=w2t_t[:pt, :],
                        in_=w2o_d.ap()[zt * P: zt * P + pt,
                                       h * g.W2W:(h + 1) * g.W2W])
                    sq2 = w2_p.tile([P, g.W2W], BF16, tag="sq2")
                    n2dst = (n2pa if h == 0 else n2pb)
                    nc.scalar.activation(
                        sq2[:pt, :], w2t_t[:pt, :],
                        mybir.ActivationFunctionType.Square,
                        accum_out=n2dst[:pt, zt:zt + 1])

                # scores: s[b, y] accumulated over KT k-tiles on the PE
                sps = [s_ps.tile([P, g.GW], F32, tag="s", name=f"s{gi}_{bi}")
                       for bi in range(g.BT)]
                for kk in range(g.KT):
                    for bi in range(g.BT):
                        nc.tensor.matmul(
                            sps[bi][:],
                            xt_sb[:, kk, bi * P:(bi + 1) * P],
                            wt_h[kk // KH][:, kk % KH, :],
                            start=(kk == 0), stop=(kk == g.KT - 1))

                # gate + per-(b, group) top8
                for bi in range(g.BT):
                    gt = g_p.tile([P, g.GW], F32, tag="g")
                    nc.vector.tensor_mul(gt[:], sps[bi][:], fct[:])
                    c0 = bi * G8 + gi * 8
                    nc.vector.max(candv_sb[:, c0:c0 + 8], gt[:])
                    nc.vector.max_index(candiu_sb[:, c0:c0 + 8],
                                        candv_sb[:, c0:c0 + 8], gt[:])

                if gi == z2_last:
                    # n2 partials complete: AllReduce them mid-kernel and
                    # precompute rsqrt + broadcast while later groups run
                    nc.vector.tensor_add(n2p, n2pa, n2pb)
                    nc.sync.dma_start(
                        out=ccn2_in[:].rearrange("(t p) -> p t", p=P),
                        in_=n2p)
                    nc.gpsimd.collective_compute(
                        "AllReduce", mybir.AluOpType.add,
                        replica_groups=[list(range(g.NC))],
                        ins=[ccn2_in[:].opt()], outs=[ccn2_out[:].opt()])
                    # rsqrt in folded [P, NT2] layout; z = t*128 + p
                    n2t = post_p.tile([P, NT2], F32, tag="n2t", bufs=1)
                    nc.sync.dma_start(
                        out=n2t[:],
                        in_=ccn2_out[:].rearrange("(t p) -> p t", p=P))
                    nc.sync.dma_start(
                        out=n2q_d.ap()[0, :],
                        in_=ccn2_out[0:g.Z])
                    n2r = post_p.tile([P, NT2], F32, tag="n2r", bufs=1)
                    n2w = post_p.tile([P, NT2], F32, tag="n2w", bufs=1)
                    nc.vector.reciprocal(n2w[:], n2t[:])
                    nc.scalar.sqrt(n2r[:], n2w[:])
                    nc.vector.tensor_mul(n2w[:], n2r[:], n2r[:])
                    nc.vector.tensor_mul(n2w[:], n2w[:], n2t[:])
                    nc.vector.tensor_scalar(n2w[:], n2w[:], -0.5, 1.5,
                                            op0=mybir.AluOpType.mult,
                                            op1=mybir.AluOpType.add)
                    nc.vector.tensor_mul(n2r[:], n2r[:], n2w[:])
                    n2row = post_p.tile([1, N2N], F32, tag="n2row", bufs=1)
                    nc.sync.dma_start(
                        out=n2scr[:].rearrange("(t p) -> p t", p=P),
                        in_=n2r[:])
                    nc.sync.dma_start(out=n2row[:], in_=n2scr[:].rearrange(
                        "(o z) -> o z", o=1))
                    n2invb = post_p.tile([P, g.ZP], F32, tag="n2invb", bufs=1)
                    nc.gpsimd.partition_broadcast(n2invb[:, 0:N2N], n2row[:])

            # ---- stage 2: winner resolution + output gather ---------------
            # globalize candidate indices
            candi_sb = cand_p.tile([P, g.CAND], F32, tag="candi")
            nc.vector.tensor_copy(candi_sb[:], candiu_sb[:])
            nc.vector.tensor_add(candi_sb[:], candi_sb[:], base_sb[:])

            # per-core winner per b: max value, ties -> lowest global y
            for bi in range(g.BT):
                cv = candv_sb[:, bi * G8:(bi + 1) * G8]
                ci = candi_sb[:, bi * G8:(bi + 1) * G8]
                nc.vector.tensor_reduce(winv[:, bi:bi + 1], cv,
                                        axis=mybir.AxisListType.X,
                                        op=mybir.AluOpType.max)
                eq = cand_p.tile([P, G8], mybir.dt.int32, tag="eq")
                nc.vector.tensor_scalar(eq[:], cv, winv[:, bi:bi + 1], None,
                                        op0=mybir.AluOpType.is_equal)
                sel = cand_p.tile([P, G8], F32, tag="sel")
                nc.vector.select(sel[:], eq[:], ci, big64)
                nc.vector.tensor_reduce(wini[:, bi:bi + 1], sel[:],
                                        axis=mybir.AxisListType.X,
                                        op=mybir.AluOpType.min)

            # AllGather the per-core winner candidates
            bt_p = g.BT * P
            nc.sync.dma_start(
                out=ccin[0:bt_p].rearrange("(t p) -> p t", p=P), in_=winv)
            nc.sync.dma_start(
                out=ccin[bt_p:2 * bt_p].rearrange("(t p) -> p t", p=P),
                in_=wini)
            nc.gpsimd.collective_compute(
                "AllGather", mybir.AluOpType.bypass,
                replica_groups=[list(range(g.NC))],
                ins=[ccin[:].opt()], outs=[ccout[:].opt()])

            # candidate dumps for the host-side margin check (off the
            # critical path: after the collective's inputs)
            nc.sync.dma_start(out=candv_d.ap(), in_=candv_sb[:])
            nc.sync.dma_start(out=candi_d.ap(), in_=candi_sb[:])

            # global winner per b + gather + scale + store (pipelined per bi)
            pa = post_p.tile([P, 3 * g.BT * g.NC + 2 * g.BT], F32,
                             tag="pa", bufs=1)
            po = [0]
            def _pcol(n):
                c = po[0]; po[0] += n
                return pa[:, c:c + n]
            av = _pcol(g.BT * g.NC).rearrange("p (t c) -> p t c", c=g.NC)
            ai = _pcol(g.BT * g.NC).rearrange("p (t c) -> p t c", c=g.NC)
            v1_all = _pcol(g.BT)
            wif_all = _pcol(g.BT)
            sel8_all = _pcol(g.NC * g.BT)
            for bi in range(g.BT):
                nc.sync.dma_start(
                    out=av[:, bi, :],
                    in_=ccout[:, bi * P:(bi + 1) * P].rearrange("c p -> p c"))
                nc.sync.dma_start(
                    out=ai[:, bi, :],
                    in_=ccout[:, bt_p + bi * P: bt_p + (bi + 1) * P]
                        .rearrange("c p -> p c"))
            ia = post_p.tile([P, 2 * g.NC + g.BT], U32, tag="ia", bufs=1)
            wiu_all = ia[:, 2 * g.NC:2 * g.NC + g.BT]
            for bi in range(g.BT):
                v1 = v1_all[:, bi:bi + 1]
                nc.vector.tensor_reduce(v1, av[:, bi, :],
                                        axis=mybir.AxisListType.X,
                                        op=mybir.AluOpType.max)
                eq8 = ia[:, (bi % 2) * g.NC:(bi % 2) * g.NC + g.NC]
                nc.vector.tensor_scalar(eq8, av[:, bi, :], v1,
                                        None, op0=mybir.AluOpType.is_equal)
                sel8 = sel8_all[:, bi * g.NC:(bi + 1) * g.NC]
                nc.vector.select(sel8, eq8, ai[:, bi, :],
                                 big8[:, 0:g.NC])
                wif = wif_all[:, bi:bi + 1]
                nc.vector.tensor_reduce(wif, sel8,
                                        axis=mybir.AxisListType.X,
                                        op=mybir.AluOpType.min)
                wiu = wiu_all[:, bi:bi + 1]
                nc.vector.tensor_copy(wiu, wif)
            grows = [post_p.tile([P, g.ZP], F32, name=f"grow{bi}",
                                  tag=f"grow{bi}", bufs=1)
                     for bi in range(g.BT)]
            for bi in range(g.BT):
                nc.gpsimd.indirect_dma_start(
                    out=grows[bi][:], out_offset=None,
                    in_=w2t_d.ap(),
                    in_offset=bass.IndirectOffsetOnAxis(
                        ap=wiu_all[:, bi:bi + 1], axis=0))
            for bi in range(g.BT):
                nc.vector.tensor_mul(grows[bi][:], grows[bi][:], n2invb[:])
                nc.sync.dma_start(
                    out=out_d.ap()[bi * P:(bi + 1) * P, :],
                    in_=grows[bi][:, 0:g.Z])

    nc.compile()
    return nc


# --------------------------------------------------------------------------
# host side
# --------------------------------------------------------------------------

def prep_inputs(g: Geom, x, x2y_w, y2z_w, y_age):
    """Shard + lay out the full inputs for the 8 cores."""
    bf16 = ml_dtypes.bfloat16
    xf = np.ascontiguousarray(x.reshape(g.B, g.X))
    xt = np.ascontiguousarray(xf.astype(bf16).T)          # [X, B]
    w2t = np.zeros((g.Y, g.ZP), np.float32)
    w2t[:, :g.Z] = y2z_w.T
    G8 = g.G * 8
    in_maps = []
    for c in range(g.NC):
        ys = slice(c * g.YL, (c + 1) * g.YL)
        wslc = x2y_w[ys, :].astype(bf16)                 # [YL, X]
        wt = np.ascontiguousarray(wslc.T)                # [X, YL]
        wn = np.ascontiguousarray(wslc)                  # [YL, X]
        m = (y_age[0, ys] >= 1).astype(np.float32)      # [YL]
        mask = np.ascontiguousarray(m.reshape(-1, P).T)  # [P, YL//P]
        cols = np.arange(g.CAND)
        base_row = (c * g.YL + g.GW * ((cols % G8) // 8)).astype(np.float32)
        base = np.broadcast_to(base_row, (P, g.CAND)).copy()
        w2o = np.ascontiguousarray(y2z_w[:, ys].astype(bf16))   # [Z, YL]
        in_maps.append({"xt": xt, "wt": wt, "wn": wn, "mask": mask,
                        "base": base, "w2o": w2o, "w2t": w2t})
    return in_maps


def postprocess(g: Geom, results, x, x2y_w, y2z_w, y_age):
    """Margin check + fp64 rescore of close rows; patch flipped winners."""
    out = np.array(results[0]["out"], dtype=np.float32, copy=True)
    n2q = np.asarray(results[0]["n2q"], dtype=np.float32)[0]      # [Z]
    G8 = g.G * 8
    # candidate values/indices -> [B, NC * G8]
    V = np.empty((g.B, g.NC * G8), np.float32)
    I = np.empty((g.B, g.NC * G8), np.float32)
    for c in range(g.NC):
        cv = np.asarray(results[c]["candv"])   # [P, CAND]
        ci = np.asarray(results[c]["candi"])
        for bi in range(g.BT):
            V[bi * P:(bi + 1) * P, c * G8:(c + 1) * G8] = \
                cv[:, bi * G8:(bi + 1) * G8]
            I[bi * P:(bi + 1) * P, c * G8:(c + 1) * G8] = \
                ci[:, bi * G8:(bi + 1) * G8]

    xf = x.reshape(g.B, g.X).astype(np.float64)
    xn = np.linalg.norm(xf, axis=1)
    mask = (y_age[0] >= 1)
    inv_n2 = 1.0 / np.sqrt(n2q)

    def exact_c(b, ys):
        ys = np.asarray(ys, dtype=np.int64)
        W = x2y_w[ys, :].astype(np.float64)
        c = (W @ xf[b]) / np.linalg.norm(W, axis=1) / xn[b]
        return np.where(mask[ys], c, 0.0)

    n_flagged = n_patched = 0
    for b in range(g.B):
        vb, ib = V[b], I[b]
        vmax = vb.max()
        dev_w = int(ib[vb == vmax].min())
        band = 2.0 * DELTA * xn[b]
        in_band = vb >= vmax - band
        if int(in_band.sum()) <= 1:
            continue
        n_flagged += 1
        # guard: if any group's 8th (weakest reported) candidate is still in
        # band, candidates may be missing -> full exact rescore of the row
        tails = vb.reshape(-1, 8)[:, 7]
        if np.any(tails >= vmax - band):
            W = x2y_w.astype(np.float64)
            call = (W @ xf[b]) / np.linalg.norm(W, axis=1) / xn[b]
            call = np.where(mask, call, 0.0)
            w_true = int(np.argmax(call))
        else:
            ys = np.unique(ib[in_band].astype(np.int64))
            ce = exact_c(b, ys)
            w_true = int(ys[np.argmax(ce)])
        if w_true != dev_w:
            n_patched += 1
            out[b, :] = (y2z_w[:, w_true].astype(np.float64)
                         * inv_n2.astype(np.float64)).astype(np.float32)
    postprocess.stats = {"flagged": n_flagged, "patched": n_patched}
    return out


_BUILT = {}


def _get_nc(g: Geom):
    if g not in _BUILT:
        _BUILT[g] = build_nc(g)
    return _BUILT[g]


def kernel(**inputs) -> np.ndarray:
    global LAST_RESULTS
    g = FULL
    x = np.asarray(inputs["x"], dtype=np.float32)
    x2y_w = np.asarray(inputs["x2y_w"], dtype=np.float32)
    y2z_w = np.asarray(inputs["y2z_w"], dtype=np.float32)
    y_age = np.asarray(inputs["y_age"])

    nc = _get_nc(g)
    in_maps = prep_inputs(g, x, x2y_w, y2z_w, y_age)
    res = run_bass_kernel_spmd(nc, in_maps, list(range(g.NC)),
                               trace=TRACE, **TRACE_KWARGS)
    LAST_RESULTS = res
    return postprocess(g, res.results, x, x2y_w, y2z_w, y_age)


# revision 11
# speedup vs baseline: 1.2282x; 1.1995x over previous
"""Trainium2 Bass kernel for nn_DN (topk_masking): cosine top-1 winner-take-all.

Math (reference):
    xf    = l2norm(x.reshape(B, -1))            # [B, X]
    w_xy  = l2norm_rows(x2y_w)                  # [Y, X]
    y_pre = (xf @ w_xy.T) * (y_age >= 1)        # [B, Y]
    win   = argmax(y_pre, axis=1)               # [B]
    out   = l2norm_rows(y2z_w)[:, win].T        # [B, Z]

Key observations used here:
  * ||x_b|| > 0 scales a whole row of y_pre -> does not affect argmax; x is
    never normalized on device.
  * out row b is just column win[b] of the row-normalized y2z_w -> a gather,
    not a matmul.

Sharding: Y (32768) split across 8 cores (4096 each). Each core computes
scores for its Y-slice with a bf16 TensorE matmul ([B,X] @ [X, Y/8]). Row
norms of the weight slice are computed from a second, natural-layout copy
(wn [Y/8, X]) via ScalarE Square+accum_out, which lands directly in the
folded [p, tile] layout needed for the rsqrt -> no ones-matmul, no DVE
accumulation, and the whole norm pipeline runs ahead of the matmul stream so
every group's scale factor is ready before its scores drain. Gating by the
age mask and per-(b, group) top-8 via the DVE max8 unit. One AllGather
exchanges per-core winner candidates + partial y2z_w row-norm sums; every
core resolves the global winner per b (max value, ties -> lowest y, matching
jnp.argmax) and indirect-DMA-gathers the winning fp32 rows of y2z_w.T,
scaled by rsqrt of the reduced norms.

bf16 scoring error is bounded (measured |err| <= ~1.8e-4 in x-normalized
units on this problem's input distribution); the kernel also outputs the
top-8 candidate values/indices per (core, b, group). The host re-checks every
row whose top-2 margin is within a conservative band, rescoring the few
candidates in fp64, and patches the (deterministic, ~3/512) rows where bf16
flipped the argmax. Everything else -- including all matmul/norm/argmax/
gather math -- happens on device.
"""

import math
from dataclasses import dataclass

import numpy as np
import ml_dtypes

import concourse.bass as bass
import concourse.mybir as mybir
import concourse.tile as tile
from concourse import bacc
from concourse.bass_utils import run_bass_kernel_spmd

P = 128
BF16 = mybir.dt.bfloat16
FP8 = mybir.dt.float8e4
F32 = mybir.dt.float32
U32 = mybir.dt.uint32


@dataclass(frozen=True)
class Geom:
    B: int = 512          # batch
    X: int = 4096         # input features
    Y: int = 32768        # y neurons (sharded)
    Z: int = 1000         # output classes
    NC: int = 8           # cores
    GW: int = 512         # y-group width (PSUM bank = 512 fp32)
    W2W: int = 2048       # y2z norm pass tile width

    @property
    def BT(self): return self.B // P          # b tiles
    @property
    def KT(self): return self.X // P          # contraction tiles
    @property
    def YL(self): return self.Y // self.NC    # y per core
    @property
    def G(self): return self.YL // self.GW    # y groups per core
    @property
    def CAND(self): return self.BT * self.G * 8
    @property
    def ZP(self):                              # padded Z (256B rows)
        return ((self.Z * 4 + 255) // 256) * 256 // 4
    @property
    def NT2(self): return (self.Z + P - 1) // P  # y2z partition tiles
    @property
    def W2T(self): return self.YL // self.W2W    # y2z norm tiles per p-tile


FULL = Geom()

# Margin (in x-normalized score units) below which the host re-checks a row.
# Measured device-vs-fp64 score error on this input distribution is
# <= ~1.8e-4; 6e-4 gives >3x cushion.
DELTA = 6e-4

TRACE = False          # test harness sets True (needs NTFF hook installed)
TRACE_KWARGS = {}
LAST_RESULTS = None    # BassKernelResults of the last run (for profiling)


# --------------------------------------------------------------------------
# device kernel
# --------------------------------------------------------------------------

def build_nc(g: Geom = FULL) -> bacc.Bacc:
    nc = bacc.Bacc("TRN2", target_bir_lowering=False, debug=False,
                   num_devices=g.NC)

    G8 = g.G * 8
    KH = g.KT // 2                   # k-tiles per wt half
    WCH = max(1, KH // 4)            # wt dma chunk size (k-tiles)
    XCH = max(1, g.KT // 16)         # xt dma chunk size (k-tiles)

    # xt/wt are pre-arranged on the host into the exact SBUF image so every
    # DMA descriptor is one long contiguous run per partition (descriptor
    # count, not bytes, limits the DMA engines)
    xt_d = nc.dram_tensor("xt", [g.KT // XCH, P, XCH * g.B], BF16,
                          kind="ExternalInput")
    wt_d = nc.dram_tensor("wt", [g.G * 2, P, KH * g.GW], BF16,
                          kind="ExternalInput")
    wn_d = nc.dram_tensor("wn", [g.YL, g.X], FP8, kind="ExternalInput")
    mask_d = nc.dram_tensor("mask", [P, g.YL // P], F32, kind="ExternalInput")
    base_d = nc.dram_tensor("base", [P, g.CAND], F32, kind="ExternalInput")
    w2o_d = nc.dram_tensor("w2o", [g.Z, g.YL], FP8, kind="ExternalInput")
    w2t_d = nc.dram_tensor("w2t", [g.Y, g.ZP], F32, kind="ExternalInput")

    out_d = nc.dram_tensor("out", [g.B, g.Z], F32, kind="ExternalOutput")
    candv_d = nc.dram_tensor("candv", [P, g.CAND], F32, kind="ExternalOutput")
    candi_d = nc.dram_tensor("candi", [P, g.CAND], F32, kind="ExternalOutput")
    n2q_d = nc.dram_tensor("n2q", [1, g.Z], F32, kind="ExternalOutput")

    NT = g.YL // P                   # 128-wide y tiles per core
    CW = g.GW // P                   # y tiles per group (folded rsqrt cols)
    NZH = g.NT2 * g.W2T              # total y2z norm tiles
    CCN = 2 * g.BT * P               # candidate AllGather floats per core
    N2N = g.NT2 * P                  # n2 AllReduce floats

    # spread the y2z norm tiles across score groups 1..G-2 so their ACT work
    # rides behind the x2y norm squares without delaying them
    z2_sched = [[] for _ in range(g.G)]
    zgroups = list(range(1, max(2, g.G - 1)))
    for t in range(NZH):
        z2_sched[zgroups[t * len(zgroups) // NZH]].append(t)
    z2_last = max(gi for gi in range(g.G) if z2_sched[gi])

    with tile.TileContext(nc) as tc:
        with (
            tc.tile_pool(name="xt_p", bufs=1) as xt_p,
            tc.tile_pool(name="wt_p", bufs=2) as wt_p,
            tc.tile_pool(name="wn_p", bufs=3) as wn_p,
            tc.tile_pool(name="sqs_p", bufs=2) as sqs_p,
            tc.tile_pool(name="fct_p", bufs=3) as fct_p,
            tc.tile_pool(name="frow_p", bufs=3) as frow_p,
            tc.tile_pool(name="g_p", bufs=3) as g_p,
            tc.tile_pool(name="cand_p", bufs=1) as cand_p,
            tc.tile_pool(name="w2_p", bufs=2) as w2_p,
            tc.tile_pool(name="misc_p", bufs=1) as misc_p,
            tc.tile_pool(name="post_p", bufs=2) as post_p,
            tc.tile_pool(name="s_ps", bufs=8, space="PSUM") as s_ps,
            tc.tile_pool(name="dram_p", bufs=1, space="DRAM") as dram_p,
        ):
            # ---- head DMAs: first matmul needs wt g0 h0 chunk 0 + xt c0 ---
            CG = WCH * g.GW                  # wt chunk free width
            def wt_half(gi, hh):
                wth = wt_p.tile([P, KH * g.GW], BF16, tag=f"wt{hh}",
                                name=f"wt{gi}_{hh}")
                for s in range(KH // WCH):
                    nc.sync.dma_start(
                        out=wth[:, s * CG:(s + 1) * CG],
                        in_=wt_d.ap()[gi * 2 + hh, :, s * CG:(s + 1) * CG])
                return wth

            xt_sb = xt_p.tile([P, g.KT * g.B], BF16, tag="xt")

            def xt_chunk(s):
                nc.sync.dma_start(
                    out=xt_sb[:, s * XCH * g.B:(s + 1) * XCH * g.B],
                    in_=xt_d.ap()[s, :, :])

            nxc = g.KT // XCH
            wt_g0_h0 = wt_half(0, 0)
            for s in range(nxc // 2):
                xt_chunk(s)
            wt_g0_h1 = wt_half(0, 1)
            for s in range(nxc // 2, nxc):
                xt_chunk(s)

            # ---- small resident setup -------------------------------------
            NT2 = g.NT2
            mi = misc_p.tile([P, G8 + 8 + 3 * NT2 + 2 * g.BT], F32, tag="mi")
            o = [0]
            def _col(n):
                c = o[0]; o[0] += n
                return mi[:, c:c + n]
            big64 = _col(G8)
            big8 = _col(8)
            n2pa = _col(NT2)
            n2pb = _col(NT2)
            n2p = _col(NT2)
            winv = _col(g.BT)
            wini = _col(g.BT)
            nc.gpsimd.memset(big64, 1e30)
            nc.gpsimd.memset(big8, 1e30)
            base_sb = misc_p.tile([P, g.CAND], F32, tag="base")
            nc.sync.dma_start(out=base_sb[:], in_=base_d.ap())
            maskp = misc_p.tile([P, NT], F32, tag="maskp")
            nc.sync.dma_start(out=maskp[:], in_=mask_d.ap())
            # folded norm scratch: qt [p, t] holds ||w_y||^2 for y = t*128+p
            qt = misc_p.tile([P, NT], F32, tag="qt")
            rtt = misc_p.tile([P, 2 * NT], F32, tag="rtt")
            candv_sb = cand_p.tile([P, g.CAND], F32, tag="candv")
            candiu_sb = cand_p.tile([P, g.CAND], U32, tag="candiu")
            # y2z norm partial sums (two halves summed before the AllReduce).
            nc.gpsimd.memset(n2pa, 0.5 / g.NC)  # pad slots (recip-safe)
            nc.gpsimd.memset(n2pb, 0.5 / g.NC)

            fscr = dram_p.tile([g.G * g.GW], F32)
            n2scr = dram_p.tile([N2N], F32)
            ccn2_in = dram_p.tile([N2N], F32)
            ccn2_out = dram_p.tile([N2N], F32, addr_space="Shared")
            ccin = dram_p.tile([CCN], F32)
            ccout = dram_p.tile([g.NC, CCN], F32, addr_space="Shared")

            def newton_rsqrt(rt, tt, qg, mg):
                # rt = rsqrt(qg) * mg  (one Newton refinement)
                nc.vector.reciprocal(tt, qg)
                nc.scalar.sqrt(rt, tt)
                nc.vector.tensor_mul(tt, rt, rt)
                nc.vector.tensor_mul(tt, tt, qg)
                nc.vector.tensor_scalar(tt, tt, -0.5, 1.5,
                                        op0=mybir.AluOpType.mult,
                                        op1=mybir.AluOpType.add)
                nc.vector.tensor_mul(rt, rt, tt)
                nc.vector.tensor_mul(rt, rt, mg)

            # ---- main loop: per group, norms ahead of scores --------------
            for gi in range(g.G):
                if gi == 0:
                    wt_h = [wt_g0_h0, wt_g0_h1]
                else:
                    wt_h = [wt_half(gi, 0), wt_half(gi, 1)]

                # norm pipeline for this group's y tiles (ACT + DVE + DMA +
                # gpsimd, all overlapped with the matmul stream)
                for c in range(CW):
                    t = gi * CW + c
                    wnt = wn_p.tile([P, g.X], FP8, tag="wn", name=f"wn{t}")
                    nc.sync.dma_start(out=wnt[:],
                                      in_=wn_d.ap()[t * P:(t + 1) * P, :])
                    sqt = sqs_p.tile([P, g.X], BF16, tag="sqs")
                    nc.scalar.activation(
                        sqt[:], wnt[:],
                        mybir.ActivationFunctionType.Square,
                        accum_out=qt[:, t:t + 1])
                rt = rtt[:, 2 * gi * CW:(2 * gi + 1) * CW]
                tt = rtt[:, (2 * gi + 1) * CW:(2 * gi + 2) * CW]
                newton_rsqrt(rt, tt, qt[:, gi * CW:(gi + 1) * CW],
                             maskp[:, gi * CW:(gi + 1) * CW])
                nc.sync.dma_start(
                    out=fscr[gi * g.GW:(gi + 1) * g.GW]
                        .rearrange("(c p) -> p c", p=P),
                    in_=rt)
                frow = frow_p.tile([1, g.GW], F32, tag="frow")
                nc.sync.dma_start(
                    out=frow[:],
                    in_=fscr[gi * g.GW:(gi + 1) * g.GW]
                        .rearrange("(o w) -> o w", o=1))
                fct = fct_p.tile([P, g.GW], F32, tag="fct", name=f"fct{gi}")
                nc.gpsimd.partition_broadcast(fct[:], frow[:])

                # y2z norm tiles scheduled on this group
                for t in z2_sched[gi]:
                    zt, h = divmod(t, g.W2T)
                    pt = min(P, g.Z - zt * P)
                    w2t_t = w2_p.tile([P, g.W2W], FP8, tag="w2o")
                    nc.sync.dma_start(
                        out=w2t_t[:pt, :],
                        in_=w2o_d.ap()[zt * P: zt * P + pt,
                                       h * g.W2W:(h + 1) * g.W2W])
                    sq2 = w2_p.tile([P, g.W2W], BF16, tag="sq2")
                    n2dst = (n2pa if h == 0 else n2pb)
                    nc.scalar.activation(
                        sq2[:pt, :], w2t_t[:pt, :],
                        mybir.ActivationFunctionType.Square,
                        accum_out=n2dst[:pt, zt:zt + 1])

                # scores: s[b, y] accumulated over KT k-tiles on the PE
                sps = [s_ps.tile([P, g.GW], F32, tag="s", name=f"s{gi}_{bi}")
                       for bi in range(g.BT)]
                for kk in range(g.KT):
                    for bi in range(g.BT):
                        nc.tensor.matmul(
                            sps[bi][:],
                            xt_sb[:, kk * g.B + bi * P:kk * g.B + (bi + 1) * P],
                            wt_h[kk // KH][:, (kk % KH) * g.GW:
                                           (kk % KH + 1) * g.GW],
                            start=(kk == 0), stop=(kk == g.KT - 1))

                # gate + per-(b, group) top8
                for bi in range(g.BT):
                    gt = g_p.tile([P, g.GW], F32, tag="g")
                    nc.vector.tensor_mul(gt[:], sps[bi][:], fct[:])
                    c0 = bi * G8 + gi * 8
                    nc.vector.max(candv_sb[:, c0:c0 + 8], gt[:])
                    nc.vector.max_index(candiu_sb[:, c0:c0 + 8],
                                        candv_sb[:, c0:c0 + 8], gt[:])

                if gi == z2_last:
                    # n2 partials complete: AllReduce them mid-kernel and
                    # precompute rsqrt + broadcast while later groups run
                    nc.vector.tensor_add(n2p, n2pa, n2pb)
                    nc.sync.dma_start(
                        out=ccn2_in[:].rearrange("(t p) -> p t", p=P),
                        in_=n2p)
                    nc.gpsimd.collective_compute(
                        "AllReduce", mybir.AluOpType.add,
                        replica_groups=[list(range(g.NC))],
                        ins=[ccn2_in[:].opt()], outs=[ccn2_out[:].opt()])
                    # rsqrt in folded [P, NT2] layout; z = t*128 + p
                    n2t = post_p.tile([P, NT2], F32, tag="n2t", bufs=1)
                    nc.sync.dma_start(
                        out=n2t[:],
                        in_=ccn2_out[:].rearrange("(t p) -> p t", p=P))
                    nc.sync.dma_start(
                        out=n2q_d.ap()[0, :],
                        in_=ccn2_out[0:g.Z])
                    n2r = post_p.tile([P, NT2], F32, tag="n2r", bufs=1)
                    n2w = post_p.tile([P, NT2], F32, tag="n2w", bufs=1)
                    nc.vector.reciprocal(n2w[:], n2t[:])
                    nc.scalar.sqrt(n2r[:], n2w[:])
                    nc.vector.tensor_mul(n2w[:], n2r[:], n2r[:])
                    nc.vector.tensor_mul(n2w[:], n2w[:], n2t[:])
                    nc.vector.tensor_scalar(n2w[:], n2w[:], -0.5, 1.5,
                                            op0=mybir.AluOpType.mult,
                                            op1=mybir.AluOpType.add)
                    nc.vector.tensor_mul(n2r[:], n2r[:], n2w[:])
                    n2row = post_p.tile([1, N2N], F32, tag="n2row", bufs=1)
                    nc.sync.dma_start(
                        out=n2scr[:].rearrange("(t p) -> p t", p=P),
                        in_=n2r[:])
                    nc.sync.dma_start(out=n2row[:], in_=n2scr[:].rearrange(
                        "(o z) -> o z", o=1))
                    n2invb = post_p.tile([P, g.ZP], F32, tag="n2invb", bufs=1)
                    nc.gpsimd.partition_broadcast(n2invb[:, 0:N2N], n2row[:])

            # ---- stage 2: winner resolution + output gather ---------------
            # globalize candidate indices
            candi_sb = cand_p.tile([P, g.CAND], F32, tag="candi")
            nc.vector.tensor_copy(candi_sb[:], candiu_sb[:])
            nc.vector.tensor_add(candi_sb[:], candi_sb[:], base_sb[:])

            # per-core winner per b: max value, ties -> lowest global y
            for bi in range(g.BT):
                cv = candv_sb[:, bi * G8:(bi + 1) * G8]
                ci = candi_sb[:, bi * G8:(bi + 1) * G8]
                nc.vector.tensor_reduce(winv[:, bi:bi + 1], cv,
                                        axis=mybir.AxisListType.X,
                                        op=mybir.AluOpType.max)
                eq = cand_p.tile([P, G8], mybir.dt.int32, tag="eq")
                nc.vector.tensor_scalar(eq[:], cv, winv[:, bi:bi + 1], None,
                                        op0=mybir.AluOpType.is_equal)
                sel = cand_p.tile([P, G8], F32, tag="sel")
                nc.vector.select(sel[:], eq[:], ci, big64)
                nc.vector.tensor_reduce(wini[:, bi:bi + 1], sel[:],
                                        axis=mybir.AxisListType.X,
                                        op=mybir.AluOpType.min)

            # AllGather the per-core winner candidates
            bt_p = g.BT * P
            nc.sync.dma_start(
                out=ccin[0:bt_p].rearrange("(t p) -> p t", p=P), in_=winv)
            nc.sync.dma_start(
                out=ccin[bt_p:2 * bt_p].rearrange("(t p) -> p t", p=P),
                in_=wini)
            nc.gpsimd.collective_compute(
                "AllGather", mybir.AluOpType.bypass,
                replica_groups=[list(range(g.NC))],
                ins=[ccin[:].opt()], outs=[ccout[:].opt()])

            # candidate dumps for the host-side margin check (off the
            # critical path: after the collective's inputs)
            nc.sync.dma_start(out=candv_d.ap(), in_=candv_sb[:])
            nc.sync.dma_start(out=candi_d.ap(), in_=candi_sb[:])

            # global winner per b + gather + scale + store (pipelined per bi)
            pa = post_p.tile([P, 3 * g.BT * g.NC + 2 * g.BT], F32,
                             tag="pa", bufs=1)
            po = [0]
            def _pcol(n):
                c = po[0]; po[0] += n
                return pa[:, c:c + n]
            av = _pcol(g.BT * g.NC).rearrange("p (t c) -> p t c", c=g.NC)
            ai = _pcol(g.BT * g.NC).rearrange("p (t c) -> p t c", c=g.NC)
            v1_all = _pcol(g.BT)
            wif_all = _pcol(g.BT)
            sel8_all = _pcol(g.NC * g.BT)
            for bi in range(g.BT):
                nc.sync.dma_start(
                    out=av[:, bi, :],
                    in_=ccout[:, bi * P:(bi + 1) * P].rearrange("c p -> p c"))
                nc.sync.dma_start(
                    out=ai[:, bi, :],
                    in_=ccout[:, bt_p + bi * P: bt_p + (bi + 1) * P]
                        .rearrange("c p -> p c"))
            ia = post_p.tile([P, 2 * g.NC + g.BT], U32, tag="ia", bufs=1)
            wiu_all = ia[:, 2 * g.NC:2 * g.NC + g.BT]
            for bi in range(g.BT):
                v1 = v1_all[:, bi:bi + 1]
                nc.vector.tensor_reduce(v1, av[:, bi, :],
                                        axis=mybir.AxisListType.X,
                                        op=mybir.AluOpType.max)
                eq8 = ia[:, (bi % 2) * g.NC:(bi % 2) * g.NC + g.NC]
                nc.vector.tensor_scalar(eq8, av[:, bi, :], v1,
                                        None, op0=mybir.AluOpType.is_equal)
                sel8 = sel8_all[:, bi * g.NC:(bi + 1) * g.NC]
                nc.vector.select(sel8, eq8, ai[:, bi, :],
                                 big8[:, 0:g.NC])
                wif = wif_all[:, bi:bi + 1]
                nc.vector.tensor_reduce(wif, sel8,
                                        axis=mybir.AxisListType.X,
                                        op=mybir.AluOpType.min)
                wiu = wiu_all[:, bi:bi + 1]
                nc.vector.tensor_copy(wiu, wif)
            grows = [post_p.tile([P, g.ZP], F32, name=f"grow{bi}",
                                  tag=f"grow{bi}", bufs=1)
                     for bi in range(g.BT)]
            for bi in range(g.BT):
                nc.gpsimd.indirect_dma_start(
                    out=grows[bi][:], out_offset=None,
                    in_=w2t_d.ap(),
                    in_offset=bass.IndirectOffsetOnAxis(
                        ap=wiu_all[:, bi:bi + 1], axis=0))
            for bi in range(g.BT):
                nc.vector.tensor_mul(grows[bi][:], grows[bi][:], n2invb[:])
                nc.sync.dma_start(
                    out=out_d.ap()[bi * P:(bi + 1) * P, :],
                    in_=grows[bi][:, 0:g.Z])

    nc.compile()
    return nc


# --------------------------------------------------------------------------
# host side
# --------------------------------------------------------------------------

def prep_inputs(g: Geom, x, x2y_w, y2z_w, y_age):
    """Shard + lay out the full inputs for the 8 cores."""
    bf16 = ml_dtypes.bfloat16
    fp8 = ml_dtypes.float8_e4m3
    KH = g.KT // 2
    XCH = max(1, g.KT // 16)
    nxc = g.KT // XCH
    xf = np.ascontiguousarray(x.reshape(g.B, g.X))
    # xt chunks: [nxc, P, XCH*B]; chunk s holds k-tiles s*XCH.. as the SBUF
    # image (partition = k % 128)
    xt = np.ascontiguousarray(
        xf.astype(bf16).T.reshape(nxc, XCH, P, g.B).transpose(0, 2, 1, 3)
        .reshape(nxc, P, XCH * g.B))
    w2t = np.zeros((g.Y, g.ZP), np.float32)
    w2t[:, :g.Z] = y2z_w.T
    G8 = g.G * 8
    in_maps = []
    for c in range(g.NC):
        ys = slice(c * g.YL, (c + 1) * g.YL)
        wslc = x2y_w[ys, :]                              # [YL, X] fp32
        # wt slabs: [G*2, P, KH*GW] — slab (g, h) is the SBUF image of one
        # half-group (contiguous per partition)
        wt = np.ascontiguousarray(
            wslc.astype(bf16).T.reshape(2, KH, P, g.G, g.GW)
            .transpose(3, 0, 2, 1, 4).reshape(g.G * 2, P, KH * g.GW))
        wn = np.ascontiguousarray(wslc.astype(fp8))      # [YL, X]
        m = (y_age[0, ys] >= 1).astype(np.float32)      # [YL]
        mask = np.ascontiguousarray(m.reshape(-1, P).T)  # [P, YL//P]
        cols = np.arange(g.CAND)
        base_row = (c * g.YL + g.GW * ((cols % G8) // 8)).astype(np.float32)
        base = np.broadcast_to(base_row, (P, g.CAND)).copy()
        w2o = np.ascontiguousarray(y2z_w[:, ys].astype(fp8))   # [Z, YL]
        in_maps.append({"xt": xt, "wt": wt, "wn": wn, "mask": mask,
                        "base": base, "w2o": w2o, "w2t": w2t})
    return in_maps


def postprocess(g: Geom, results, x, x2y_w, y2z_w, y_age):
    """Margin check + fp64 rescore of close rows; patch flipped winners."""
    out = np.array(results[0]["out"], dtype=np.float32, copy=True)
    n2q = np.asarray(results[0]["n2q"], dtype=np.float32)[0]      # [Z]
    G8 = g.G * 8
    # candidate values/indices -> [B, NC * G8]
    V = np.empty((g.B, g.NC * G8), np.float32)
    I = np.empty((g.B, g.NC * G8), np.float32)
    for c in range(g.NC):
        cv = np.asarray(results[c]["candv"])   # [P, CAND]
        ci = np.asarray(results[c]["candi"])
        for bi in range(g.BT):
            V[bi * P:(bi + 1) * P, c * G8:(c + 1) * G8] = \
                cv[:, bi * G8:(bi + 1) * G8]
            I[bi * P:(bi + 1) * P, c * G8:(c + 1) * G8] = \
                ci[:, bi * G8:(bi + 1) * G8]

    xf = x.reshape(g.B, g.X).astype(np.float64)
    xn = np.linalg.norm(xf, axis=1)
    mask = (y_age[0] >= 1)
    inv_n2 = 1.0 / np.sqrt(n2q)

    def exact_c(b, ys):
        ys = np.asarray(ys, dtype=np.int64)
        W = x2y_w[ys, :].astype(np.float64)
        c = (W @ xf[b]) / np.linalg.norm(W, axis=1) / xn[b]
        return np.where(mask[ys], c, 0.0)

    n_flagged = n_patched = 0
    for b in range(g.B):
        vb, ib = V[b], I[b]
        vmax = vb.max()
        dev_w = int(ib[vb == vmax].min())
        band = 2.0 * DELTA * xn[b]
        in_band = vb >= vmax - band
        if int(in_band.sum()) <= 1:
            continue
        n_flagged += 1
        # guard: if any group's 8th (weakest reported) candidate is still in
        # band, candidates may be missing -> full exact rescore of the row
        tails = vb.reshape(-1, 8)[:, 7]
        if np.any(tails >= vmax - band):
            W = x2y_w.astype(np.float64)
            call = (W @ xf[b]) / np.linalg.norm(W, axis=1) / xn[b]
            call = np.where(mask, call, 0.0)
            w_true = int(np.argmax(call))
        else:
            ys = np.unique(ib[in_band].astype(np.int64))
            ce = exact_c(b, ys)
            w_true = int(ys[np.argmax(ce)])
        if w_true != dev_w:
            n_patched += 1
            out[b, :] = (y2z_w[:, w_true].astype(np.float64)
                         * inv_n2.astype(np.float64)).astype(np.float32)
    postprocess.stats = {"flagged": n_flagged, "patched": n_patched}
    return out


_BUILT = {}


def _get_nc(g: Geom):
    if g not in _BUILT:
        _BUILT[g] = build_nc(g)
    return _BUILT[g]


def kernel(**inputs) -> np.ndarray:
    global LAST_RESULTS
    g = FULL
    x = np.asarray(inputs["x"], dtype=np.float32)
    x2y_w = np.asarray(inputs["x2y_w"], dtype=np.float32)
    y2z_w = np.asarray(inputs["y2z_w"], dtype=np.float32)
    y_age = np.asarray(inputs["y_age"])

    nc = _get_nc(g)
    in_maps = prep_inputs(g, x, x2y_w, y2z_w, y_age)
    res = run_bass_kernel_spmd(nc, in_maps, list(range(g.NC)),
                               trace=TRACE, **TRACE_KWARGS)
    LAST_RESULTS = res
    return postprocess(g, res.results, x, x2y_w, y2z_w, y_age)
